# revision 17
# baseline (speedup 1.0000x reference)
"""Block-diagonal linear layer (8 x [256,256] blocks) on 8 Trainium2 cores.

out = block_diag(blocks) @ inp,  inp [2048, 16384] f32, blocks [8, 256, 256] f32.

Sharding: data-parallel over the batch (column) axis — each core gets
inp[:, c*2048:(c+1)*2048] plus all the (tiny) weights, computes its
[2048, 2048] output slab, and the host concatenates the slabs.

Numerics: inputs and weights are cast to fp16 on the host (10-bit mantissa,
randn-scale data: ~3e-4 relative error), matmuls accumulate in fp32 PSUM,
and results are evicted to fp16 (another ~1e-4) and upcast to f32 on the
host. Measured end-to-end relative L2 error ~4e-4.

Layout: the host packs each core's input into the exact SBUF layout
x[p, (n*2+k)*2048 + b] = inp[n*256 + k*128 + p, c*2048 + b], so every load
DMA is [128 partitions x 8 KiB contiguous] (4 KiB-run DMAs only reach
~215 GB/s; 8 KiB runs ~330 GB/s). Outputs are packed symmetrically
y[p, (n*2+mi)*2048 + b] = out[n*256 + mi*128 + p, c*2048 + b] and unpacked
on the host.

Per-core kernel: weights resident in SBUF; per block n: one 1 MiB load on
the SP HWDGE ring, 16 LDWEIGHTS+MATMUL (fp16, N=512) into PSUM, fp32->fp16
PSUM evictions on the Vector engine, one 1 MiB store on the Activation
HWDGE ring (separate rings so loads and stores never head-of-line block
each other).

Toolchain notes baked into this design:
- nc must be a bacc.Bacc (not bass.Bass): walrus here allows ONE semaphore
  wait per instruction, and Bacc.compile()'s generate_event_semaphores pass
  splits excess waits into EventSemaphore instructions.
"""

import numpy as np

N_BLOCKS = 8
D = 256           # block dim
N = N_BLOCKS * D  # 2048
BATCH = 16384
NCORES = 8
BS = BATCH // NCORES  # per-core batch shard: 2048
P = 128
FREE = 512        # matmul moving free dim (= one fp32 PSUM bank)
NJ = BS // FREE   # matmul chunks per slab: 4

_CACHE = {}


def _build_packed(mm_dtype_name: str = "float16"):
    import concourse.bacc as bacc
    import concourse.mybir as mybir
    import concourse.tile as tile

    mm_dt = getattr(mybir.dt, mm_dtype_name)
    nc = bacc.Bacc()
    # x[p, (n*2+k)*BS + b] = inp[n*256 + k*128 + p, b]  (host-packed)
    inp = nc.declare_dram_parameter("inp", [P, 2 * N_BLOCKS * BS], mm_dt, isOutput=False)
    # wt[n] = blocks[n].T  (host pre-transposed so lhsT tiles are contiguous)
    wt = nc.declare_dram_parameter("wt", [N_BLOCKS, D, D], mm_dt, isOutput=False)
    # y[p, (n*2+mi)*BS + b] = out[n*256 + mi*128 + p, b]  (host-unpacked)
    out = nc.declare_dram_parameter("out", [P, 2 * N_BLOCKS * BS], mm_dt, isOutput=True)

    with tile.TileContext(nc) as tc:
        with (
            tc.tile_pool(name="w", bufs=1) as wpool,
            tc.tile_pool(name="x", bufs=8) as xpool,
            tc.tile_pool(name="y", bufs=4) as ypool,
            tc.tile_pool(name="ps", bufs=4, space="PSUM") as pspool,
        ):
            # PE warmup: the HAM clock gate keeps the PE at 1.2 GHz until it
            # has been busy ~3.4us. Run dep-free dummy matmuls on zeroed
            # tiles while the first loads are in flight so the real matmuls
            # start (and stay) at 2.4 GHz.
            warm_w = wpool.tile([P, P], mm_dt, tag="warmw")
            warm_x = wpool.tile([P, FREE], mm_dt, tag="warmx")
            nc.any.memset(warm_w[:], 0.0)
            nc.any.memset(warm_x[:], 0.0)
            warm_ps = pspool.tile([P, FREE], mybir.dt.float32, tag="ps")
            N_WARM = 16
            for i in range(N_WARM):
                nc.tensor.matmul(
                    warm_ps[:], warm_w[:], warm_x[:],
                    start=(i == 0), stop=(i == N_WARM - 1),
                )

            # All weights resident in SBUF: [128, 8*2*256] = 8 KiB/partition.
            # Column block (n*2+k)*256 + mi*128 holds lhsT for (block n,
            # K-tile k, out-row-half mi): w_all[p, ...] = wt[n, k*128+p, mi*128+m].
            w_all = wpool.tile([P, N_BLOCKS * 2 * D], mm_dt)
            nc.sync.dma_start(
                out=w_all[:].rearrange("p (s f) -> p s f", f=D),
                in_=wt[:].rearrange("n (k p) f -> p (n k) f", p=P),
            )

            for n in range(N_BLOCKS):
                xt = xpool.tile([P, 2 * BS], mm_dt, tag="x")
                if n == 0:
                    # Split the first load so the k=0 matmuls start half a
                    # transfer earlier.
                    nc.sync.dma_start(out=xt[:, :BS], in_=inp[:, :BS])
                    nc.sync.dma_start(out=xt[:, BS:], in_=inp[:, BS : 2 * BS])
                else:
                    nc.sync.dma_start(
                        out=xt[:], in_=inp[:, (2 * n) * BS : (2 * n + 2) * BS]
                    )
                yt = ypool.tile([P, 2 * BS], mm_dt, tag="y")
                for mi in range(2):
                    # Two 2-bank PSUM tiles per mi; matmul outputs slice into
                    # single banks, evictions cover both banks in one op.
                    pss = [pspool.tile([P, 2 * FREE], mybir.dt.float32, tag="ps",
                                       name=f"ps_{n}_{mi}_{h}")
                           for h in range(2)]
                    # k outer: 4 consecutive matmuls share the same stationary
                    # weights, and psum accumulation groups interleave across
                    # the 4 banks so fills and drains overlap.
                    for k in range(2):
                        col = (n * 2 + k) * D + mi * P
                        for j in range(NJ):
                            nc.tensor.matmul(
                                pss[j // 2][:, (j % 2) * FREE : (j % 2 + 1) * FREE],
                                w_all[:, col : col + P],
                                xt[:, k * BS + j * FREE : k * BS + (j + 1) * FREE],
                                start=(k == 0),
                                stop=(k == 1),
                                skip_group_check=True,
                            )
                    for h in range(2):
                        dst = yt[:, mi * BS + 2 * h * FREE : mi * BS + 2 * (h + 1) * FREE]
                        # Alternate PSUM evictions between Vector and Scalar so
                        # neither engine's copy chain gates the stores.
                        if h == 0:
                            nc.vector.tensor_copy(dst, pss[h][:])
                        else:
                            nc.scalar.copy(dst, pss[h][:])
                # Stores ride the Activation HWDGE ring so they don't
                # head-of-line-block the loads on the SP ring.
                if n == N_BLOCKS - 1:
                    # Split the last store so the final completion semaphore
                    # (which gates the kernel-exit drain) fires earlier.
                    nc.scalar.dma_start(
                        out=out[:, (2 * n) * BS : (2 * n + 1) * BS], in_=yt[:, :BS]
                    )
                    nc.scalar.dma_start(
                        out=out[:, (2 * n + 1) * BS : (2 * n + 2) * BS], in_=yt[:, BS:]
                    )
                else:
                    nc.scalar.dma_start(
                        out=out[:, (2 * n) * BS : (2 * n + 2) * BS], in_=yt[:]
                    )
    nc.compile()
    return nc


def _get_nc(key: str):
    if key not in _CACHE:
        _CACHE[key] = _build_packed(key)
    return _CACHE[key]


LAST_RESULTS = None  # BassKernelResults of the most recent run (for test.py)


def kernel(inp: np.ndarray, blocks: np.ndarray, _trace: bool = False,
           _mm_dtype: str = "float16") -> np.ndarray:
    global LAST_RESULTS
    import concourse.mybir as mybir
    from concourse.bass_utils import run_bass_kernel_spmd

    nc = _get_nc(_mm_dtype)
    np_dt = mybir.dt.np(getattr(mybir.dt, _mm_dtype))

    inp = np.asarray(inp, dtype=np.float32)
    blocks = np.asarray(blocks, dtype=np.float32)
    # pack: v[n, k, p, c, b] = inp[n*256 + k*128 + p, c*2048 + b]
    v = inp.reshape(N_BLOCKS, 2, P, NCORES, BS).astype(np_dt)
    # x_packed[c, p, ((n*2+k))*BS + b]
    x_packed = np.ascontiguousarray(v.transpose(3, 2, 0, 1, 4).reshape(NCORES, P, -1))
    wt = np.ascontiguousarray(blocks.transpose(0, 2, 1)).astype(np_dt)

    in_maps = [{"inp": x_packed[c], "wt": wt} for c in range(NCORES)]
    res = run_bass_kernel_spmd(nc, in_maps, core_ids=list(range(NCORES)), trace=_trace)
    LAST_RESULTS = res
    # unpack: y[c][p, (n*2+mi)*BS + b] -> out[n*256 + mi*128 + p, c*2048 + b]
    y = np.stack([res.results[c]["out"] for c in range(NCORES)])  # [c, p, S*BS]
    y = y.reshape(NCORES, P, N_BLOCKS, 2, BS).astype(np.float32)
    out = y.transpose(2, 3, 1, 0, 4).reshape(N, BATCH)
    return np.ascontiguousarray(out)
